# revision 21
# baseline (speedup 1.0000x reference)
"""Dynamic per-sample 3D Gaussian blur on 8 NeuronCores.

Sharding: pure data parallelism over (batch=4) x (channel=2) -> 8 cores,
one [160,160,160] volume per core. Per core the separable blur runs as
three banded-matmul passes on the TensorEngine; the stationary operand is
the data chunk so each pass also rotates the layout for the next axis:

  pass1 (D): x0[d', (w,h)] -> psum[h, d]   @ fixed w -> x1[h, (w,d)]
  pass2 (H): x1[h', (w,d)] -> psum[w, h]   @ fixed d -> x2[w, (d,h)]
  pass3 (W): x2[w', (d,h)] -> psum[(d,h), w] -> staging -> HBM (bf16)

Key layout/packing decisions vs the naive version:
 - Input is cast to bf16 AND transposed to (d, w, h) on the host: input
   DMA is half the bytes, needs no on-chip cast, and pass-1 chunks at
   fixed w depend on one contiguous 160-col slab -> the input streams in
   while pass 1 computes.
 - Output is written bf16 and up-cast on the host (half the output DMA).
 - The conv-axis is contracted as K=128 (main) + K=32 (tail) matmuls.
   All M=32 matmuls (the h/w tail rows of each pass) are packed 4-wide
   into the PE's 32-col subarray tiles via tile_position, so 4 chunks'
   tails stream concurrently.  Tail tensors are stored in 32-aligned
   partition stripes (stripe = w%...32-block / d%4) so every psum->SBUF
   copy is one lane-aligned [128, 480] copy.
 - The gaussian band tail block gb (T[128:160, 122:160]) is replicated
   on all four partition quadrants on host so each row-group matmul has
   a partition-matched rhs.
"""

import os
from contextlib import ExitStack

import numpy as np
import ml_dtypes

_VARIANT = os.environ.get("BLUR_VARIANT", "full")  # debug bisect knob

import concourse.bass as bass
import concourse.tile as tile
from concourse import bacc, mybir
from concourse.bass_utils import run_bass_kernel_spmd

N = 160            # cube edge
S = N * N          # 25600 spatial positions per pass
NB = 13            # gaussian window
HALF = 6
A_N = 134          # big block out-cols [0, 134): windows within rows [0,128)
B_C0 = 122         # small block out-cols [122, 160): windows touching rows [128,160)
B_N = N - B_C0     # 38
GB_C0 = 3 * A_N    # col offset of the gb blocks in the packed G tile
G_COLS = GB_C0 + 3 * B_N  # 402 + 114 = 516
EPS = 1e-7
GRP = 9            # chunks per pa psum tile (3 banks, 3 chunks/bank)
BANK = 512         # psum bank capacity in f32
SLAB = 3 * N       # 480: input dma slab = one pa bank's columns

BF16 = ml_dtypes.bfloat16
F32 = mybir.dt.float32
BF = mybir.dt.bfloat16

_PROGRAM = None


def _gaussian_1d(sigma):
    loc = (np.arange(NB, dtype=np.float32) - np.float32((NB - 1) / 2.0))
    s = np.float32(sigma)
    g = np.exp(-(loc * loc) / (2.0 * s * s + np.float32(EPS))
               - np.log(np.sqrt(np.float32(2.0 * np.pi)) * s + np.float32(EPS)))
    g = g.astype(np.float32)
    return g / g.sum(dtype=np.float32)


def _band(g):
    # T[r, c] = g[r - c + HALF] on the band, zero elsewhere ('SAME' zero pad)
    t = np.zeros((N, N), np.float32)
    for k in range(NB):
        off = k - HALF  # r = c + off
        c0 = max(0, -off)
        c1 = min(N, N - off)
        idx = np.arange(c0, c1)
        t[idx + off, idx] = g[k]
    return t


def _gpack(sigma_row):
    """[128, G_COLS] bf16: cols [p*134,(p+1)*134) = T_p[0:128, 0:134];
    cols [402+38p : 402+38(p+1)) = T_p[128:160, 122:160] replicated on
    all four 32-partition quadrants."""
    out = np.zeros((128, G_COLS), np.float32)
    for p in range(3):
        t = _band(_gaussian_1d(sigma_row[p]))
        out[:, p * A_N:(p + 1) * A_N] = t[0:128, 0:A_N]
        gb = t[128:N, B_C0:N]
        for m in range(4):
            out[32 * m:32 * (m + 1), GB_C0 + p * B_N:GB_C0 + (p + 1) * B_N] = gb
    return out.astype(BF16)


class _Emitter:
    """Round-robin psum->SBUF copies between DVE and ACT."""

    def __init__(self, nc):
        self.nc = nc
        self.flip = 0

    def copy(self, dst, src):
        if self.flip % 2 == 0:
            self.nc.vector.tensor_copy(dst, src)
        else:
            self.nc.scalar.copy(dst, src)
        self.flip += 1


def _emit_bank(nc, mms):
    """mms: ordered list of (out_ap, lhsT, rhs, tile_position) written to
    ONE psum bank. start=True zeroes the 2KB bank region ONLY on the
    partitions the matmul writes, so the first MM touching each partition
    range carries start; emission order must make that first MM cover the
    range fully (bigs before tails). Last MM gets stop=True."""
    n = len(mms)
    started = [False] * 128
    for i, (out, lhsT, rhs, tp) in enumerate(mms):
        b = tp[1]  # out base partition == tile col position (can't call
        m = out.partition_size()  # base_partition(): it asserts on 96)
        start = not any(started[b:b + m])
        if start:
            for p in range(b, b + m):
                started[p] = True
        nc.tensor.matmul(out, lhsT, rhs, start=start, stop=(i == n - 1),
                         skip_group_check=True, tile_position=tp)


def _build_kernel(ctx, tc, xm_in, xt_in, g_in, y_out):
    nc = tc.nc
    em = _Emitter(nc)

    gpool = ctx.enter_context(tc.tile_pool(name="g", bufs=1))
    big = ctx.enter_context(tc.tile_pool(name="big", bufs=1))
    stage = ctx.enter_context(tc.tile_pool(name="st", bufs=3))
    ps1 = ctx.enter_context(tc.tile_pool(name="ps1", bufs=5, space="PSUM"))
    ps2 = ctx.enter_context(tc.tile_pool(name="ps2", bufs=3, space="PSUM"))

    gtile = gpool.tile([128, G_COLS], BF)
    nc.sync.dma_start(gtile[:], g_in)

    def ga(p):  # [128, 134] rhs, base partition 0
        return gtile[:, (p - 1) * A_N:p * A_N]

    def gb(p, m):  # [32, 38] rhs at partition quadrant m
        return gtile[32 * m:32 * (m + 1),
                     GB_C0 + (p - 1) * B_N:GB_C0 + p * B_N]

    # persistent volume tiles
    x0m = big.tile([128, S], BF, tag="sA")    # x0 main rows d' 0:128, (w,h)
    x1m = big.tile([128, S], BF, tag="sB")    # x1 main rows h' 0:128, (w,d)
    # tailsA: parts[0:32) = x0 tail rows d'' (w,h) all cols;
    #         parts[32:64) = x1 corner (h'' x w'') cols [0:5120) = (w'', d)
    tailsA = big.tile([128, S], BF, tag="sT")
    # x1 tail rows h'', striped: part 32j+h'' , col (w%32)*160+d, j = w//32
    tailsB = big.tile([128, 32 * N], BF, tag="sU")
    # x2 tail rows w'', striped: part 32(d%4)+w'', col (d//4)*160+h
    x2t = big.tile([128, 40 * N], BF, tag="sX")

    # ---- stream input (bf16, no cast needed)
    for g0 in range(0, S, SLAB):
        sl = min(SLAB, S - g0)
        nc.sync.dma_start(x0m[:, g0:g0 + sl], xm_in[:, g0:g0 + sl])
        nc.sync.dma_start(tailsA[0:32, g0:g0 + sl], xt_in[:, g0:g0 + sl])

    # ================= pass 1: conv along d, chunks = fixed w =============
    def p1_pa_group(w0, glen):
        # one psum BANK per <=3 chunks; copy straight out (deep psum pipeline)
        pa = ps1.tile([128, BANK], F32, tag="pa", name=f"p1a_{w0}",
                      uniquify=True)
        mms = []
        for j in range(glen):
            o = j * N
            w = w0 + j
            mms.append((pa[0:128, o:o + A_N],
                        x0m[:, w * N:w * N + 128], ga(1), (0, 0)))
        for j in range(glen):
            o = j * N
            w = w0 + j
            mms.append((pa[0:128, o + B_C0:o + N],
                        tailsA[0:32, w * N:w * N + 128], gb(1, 0), (0, 0)))
        _emit_bank(nc, mms)
        em.copy(x1m[:, w0 * N:(w0 + glen) * N], pa[0:128, 0:glen * N])

    def p1_pb_bank(r0, rl):
        # rounds r0..r0+rl-1; round r packs w = 32j+r (j=0..3) into col grps
        pb = ps2.tile([128, BANK], F32, tag="pb", name=f"p1b_{r0}",
                      uniquify=True)
        mms = []
        for r in range(r0, r0 + rl):
            o = (r - r0) * N
            for j in range(4):
                w = 32 * j + r
                mms.append((pb[32 * j:32 * j + 32, o:o + A_N],
                            x0m[:, w * N + 128:w * N + N], ga(1), (0, 32 * j)))
            for j in range(4):
                w = 32 * j + r
                mms.append((pb[32 * j:32 * j + 32, o + B_C0:o + N],
                            tailsA[0:32, w * N + 128:w * N + N], gb(1, 0),
                            (0, 32 * j)))
        _emit_bank(nc, mms)
        em.copy(tailsB[:, r0 * N:(r0 + rl) * N], pb[0:128, 0:rl * N])

    def p1_corner_bank(c0, cl):
        # chunks w = 128+c (c0..c0+cl-1): out h'' tail of w-tail columns
        pc = ps2.tile([128, BANK], F32, tag="pb", name=f"p1c_{c0}",
                      uniquify=True)
        mms = []
        for c in range(c0, c0 + cl):
            o = (c - c0) * N
            w = 128 + c
            mms.append((pc[32:64, o:o + A_N],
                        x0m[:, w * N + 128:w * N + N], ga(1), (0, 32)))
            mms.append((pc[32:64, o + B_C0:o + N],
                        tailsA[0:32, w * N + 128:w * N + N], gb(1, 0),
                        (0, 32)))
        _emit_bank(nc, mms)
        em.copy(tailsA[32:64, c0 * N:(c0 + cl) * N], pc[32:64, 0:cl * N])

    pb_next = 0    # 32 rounds total, banks of 3
    cor_next = 0   # 32 corner chunks, banks of 3
    for w0 in range(0, N, 3):
        glen = min(3, N - w0)
        p1_pa_group(w0, glen)
        w_end = w0 + glen - 1
        # pb bank (r0..r0+2) feasible once w = 96+r0+2 is loaded/used
        while pb_next < 32 and 96 + min(pb_next + 2, 31) <= w_end:
            rl = min(3, 32 - pb_next)
            p1_pb_bank(pb_next, rl)
            pb_next += rl
        while cor_next < 32 and 128 + min(cor_next + 2, 31) <= w_end:
            cl = min(3, 32 - cor_next)
            p1_corner_bank(cor_next, cl)
            cor_next += cl
    while pb_next < 32:
        rl = min(3, 32 - pb_next)
        p1_pb_bank(pb_next, rl)
        pb_next += rl
    while cor_next < 32:
        cl = min(3, 32 - cor_next)
        p1_corner_bank(cor_next, cl)
        cor_next += cl

    if _VARIANT == "p1":
        nc.sync.dma_start(y_out[0:128, :], x1m[:, 0:N])
        return

    # x2 main reuses x0's slot (x0 fully consumed by pass 1)
    x2m = big.tile([128, S], BF, tag="sA")

    # ================= pass 2: conv along h, chunks = fixed d =============
    x1v = x1m[:].rearrange("p (w d) -> p w d", d=N)
    tBv = tailsB[:].rearrange("p (s d) -> p s d", d=N)
    cnv = tailsA[:].rearrange("p (s d) -> p s d", d=N)  # corner view

    def p2_pa_group(d0, glen):
        pa = ps1.tile([128, BANK], F32, tag="pa", name=f"p2a_{d0}",
                      uniquify=True)
        mms = []
        for j in range(glen):
            o = j * N
            d = d0 + j
            mms.append((pa[0:128, o:o + A_N],
                        x1v[:, 0:128, d], ga(2), (0, 0)))
        for j in range(glen):
            o = j * N
            d = d0 + j
            for m in range(4):
                mms.append((pa[32 * m:32 * m + 32, o + B_C0:o + N],
                            tBv[32 * m:32 * m + 32, 0:32, d], gb(2, m),
                            (32 * m, 32 * m)))
        _emit_bank(nc, mms)
        em.copy(x2m[:, d0 * N:(d0 + glen) * N], pa[0:128, 0:glen * N])

    def p2_pb_bank(q0, ql):
        # quads q0..q0+ql-1; quad q packs d = 4q+j (j=0..3) into col grps
        pb = ps2.tile([128, BANK], F32, tag="pb", name=f"p2b_{q0}",
                      uniquify=True)
        mms = []
        for q in range(q0, q0 + ql):
            o = (q - q0) * N
            for j in range(4):
                d = 4 * q + j
                mms.append((pb[32 * j:32 * j + 32, o:o + A_N],
                            x1v[:, 128:160, d], ga(2), (0, 32 * j)))
            for j in range(4):
                d = 4 * q + j
                mms.append((pb[32 * j:32 * j + 32, o + B_C0:o + N],
                            cnv[32:64, 0:32, d], gb(2, 1), (32, 32 * j)))
        _emit_bank(nc, mms)
        em.copy(x2t[:, q0 * N:(q0 + ql) * N], pb[0:128, 0:ql * N])

    # ================= pass 3: conv along w, chunks = 128-col (d,h) blocks
    x2tv = x2t[:].rearrange("p (q h) -> p q h", h=N)
    yv = y_out.rearrange("(k p) w -> p k w", p=128)
    NK = S // 128  # 200

    def p3_group(k0, glen):
        # NOTE: all tail pieces are M=32: two adjacent K=32 matmuls at
        # different row groups with M>=64 hard-crash the PE (LDWEIGHTS
        # pull-ahead across row groups); M=32 cross-row adjacency is the
        # pattern pass-2's diagonal tails use in production and is safe.
        ps = ps1.tile([128, BANK], F32, tag="pa", name=f"p3_{k0}",
                      uniquify=True)
        mms = []
        for j in range(glen):
            o = j * N
            k = k0 + j
            mms.append((ps[0:128, o:o + A_N],
                        x2m[:, k * 128:k * 128 + 128], ga(3), (0, 0)))
            c = k * 128
            for pa_ in range(0, 128, 32):
                d = (c + pa_) // N
                m = d % 4
                q = d // 4
                hh = c + pa_ - d * N
                mms.append((ps[pa_:pa_ + 32, o + B_C0:o + N],
                            x2tv[32 * m:32 * m + 32, q, hh:hh + 32],
                            gb(3, m), (32 * m, pa_)))
        _emit_bank(nc, mms)
        st = stage.tile([128, 3 * N], BF, tag="st", name=f"st_{k0}",
                        uniquify=True)
        em.copy(st[:, 0:glen * N], ps[0:128, 0:glen * N])
        stv = st[:].rearrange("p (k w) -> p k w", w=N)
        nc.sync.dma_start(yv[0:128, k0:k0 + glen, :], stv[:, 0:glen, :])

    # pass 2 with pass-3 groups interleaved as soon as their x2 cols exist
    p3_enabled = _VARIANT not in ("p2",)
    q_next = 0   # pb2 quads (40), banks of 3
    k_next = 0   # pass-3 blocks (200), banks of 3
    for d0 in range(0, N, 3):
        glen = min(3, N - d0)
        p2_pa_group(d0, glen)
        d_end = d0 + glen - 1
        while q_next < 40 and 4 * min(q_next + 2, 39) + 3 <= d_end:
            ql = min(3, 40 - q_next)
            p2_pb_bank(q_next, ql)
            q_next += ql
        while p3_enabled and k_next < NK:
            gl = min(3, NK - k_next)
            dmax = (128 * (k_next + gl) - 1) // N
            if dmax <= d_end and dmax <= 4 * q_next - 1:
                p3_group(k_next, gl)
                k_next += gl
            else:
                break
    while q_next < 40:
        ql = min(3, 40 - q_next)
        p2_pb_bank(q_next, ql)
        q_next += ql
    if not p3_enabled:
        nc.sync.dma_start(y_out[0:128, :], x2m[:, 0:N])
        return
    while k_next < NK:
        gl = min(3, NK - k_next)
        p3_group(k_next, gl)
        k_next += gl


def _build_program():
    global _PROGRAM
    if _PROGRAM is not None:
        return _PROGRAM
    nc = bacc.Bacc("TRN2", target_bir_lowering=False, debug=False,
                   num_devices=8)
    xm_in = nc.dram_tensor("xm_in", [128, S], BF, kind="ExternalInput").ap()
    xt_in = nc.dram_tensor("xt_in", [32, S], BF, kind="ExternalInput").ap()
    g_in = nc.dram_tensor("g_in", [128, G_COLS], BF, kind="ExternalInput").ap()
    y_out = nc.dram_tensor("y_out", [S, N], BF, kind="ExternalOutput").ap()
    with tile.TileContext(nc) as tc, ExitStack() as ctx:
        _build_kernel(ctx, tc, xm_in, xt_in, g_in, y_out)
    nc.compile()
    _PROGRAM = nc
    return nc


def _run(image, sigma, **spmd_kwargs):
    nc = _build_program()
    B, _, _, _, C = image.shape
    in_maps = []
    for core in range(8):
        b, c = divmod(core, C)
        # (d, h, w) -> (d, w, h), bf16
        vol = np.ascontiguousarray(
            image[b, :, :, :, c].transpose(0, 2, 1)).astype(BF16)
        in_maps.append({
            "xm_in": np.ascontiguousarray(vol[0:128].reshape(128, S)),
            "xt_in": np.ascontiguousarray(vol[128:N].reshape(32, S)),
            "g_in": _gpack(sigma[b]),
        })
    res = run_bass_kernel_spmd(nc, in_maps, list(range(8)), **spmd_kwargs)
    out = np.empty((B, N, N, N, C), np.float32)
    for core in range(8):
        b, c = divmod(core, C)
        y = res.results[core]["y_out"].astype(np.float32)
        out[b, :, :, :, c] = y.reshape(N, N, N)
    return out, res


def kernel(image, sigma):
    image = np.asarray(image, dtype=np.float32)
    sigma = np.asarray(sigma, dtype=np.float32)
    out, _ = _run(image, sigma)
    return out


# revision 22
# speedup vs baseline: 1.1034x; 1.1034x over previous
"""Dynamic per-sample 3D Gaussian blur on 8 NeuronCores.

Sharding: pure data parallelism over (batch=4) x (channel=2) -> 8 cores,
one [160,160,160] volume per core. Per core the separable blur runs as
three banded-matmul passes on the TensorEngine; the stationary operand is
the data chunk so each pass also rotates the layout for the next axis:

  pass1 (D): x0[d', (w,h)] -> psum[h, d]   @ fixed w -> x1[h, (w,d)]
  pass2 (H): x1[h', (w,d)] -> psum[w, h]   @ fixed d -> x2[w, (d,h)]
  pass3 (W): x2[w', (d,h)] -> psum[(d,h), w] -> staging -> HBM (bf16)

Key layout/packing decisions vs the naive version:
 - Input is cast to bf16 AND transposed to (d, w, h) on the host: input
   DMA is half the bytes, needs no on-chip cast, and pass-1 chunks at
   fixed w depend on one contiguous 160-col slab -> the input streams in
   while pass 1 computes.
 - Output is written bf16 and up-cast on the host (half the output DMA).
 - The conv-axis is contracted as K=128 (main) + K=32 (tail) matmuls.
   All M=32 matmuls (the h/w tail rows of each pass) are packed 4-wide
   into the PE's 32-col subarray tiles via tile_position, so 4 chunks'
   tails stream concurrently.  Tail tensors are stored in 32-aligned
   partition stripes (stripe = w%...32-block / d%4) so every psum->SBUF
   copy is one lane-aligned [128, 480] copy.
 - The gaussian band tail block gb (T[128:160, 122:160]) is replicated
   on all four partition quadrants on host so each row-group matmul has
   a partition-matched rhs.
"""

import os
from contextlib import ExitStack

import numpy as np
import ml_dtypes

_VARIANT = os.environ.get("BLUR_VARIANT", "full")  # debug bisect knob

import concourse.bass as bass
import concourse.tile as tile
from concourse import bacc, mybir
from concourse.bass_utils import run_bass_kernel_spmd

N = 160            # cube edge
S = N * N          # 25600 spatial positions per pass
NB = 13            # gaussian window
HALF = 6
A_N = 134          # big block out-cols [0, 134): windows within rows [0,128)
B_C0 = 122         # small block out-cols [122, 160): windows touching rows [128,160)
B_N = N - B_C0     # 38
GB_C0 = 3 * A_N    # col offset of the gb blocks in the packed G tile
G_COLS = GB_C0 + 3 * B_N  # 402 + 114 = 516
EPS = 1e-7
GRP = 9            # chunks per pa psum tile (3 banks, 3 chunks/bank)
BANK = 512         # psum bank capacity in f32
SLAB = GRP * N     # 1440: input dma slab = one pa group's columns

BF16 = ml_dtypes.bfloat16
F32 = mybir.dt.float32
BF = mybir.dt.bfloat16

_PROGRAM = None


def _gaussian_1d(sigma):
    loc = (np.arange(NB, dtype=np.float32) - np.float32((NB - 1) / 2.0))
    s = np.float32(sigma)
    g = np.exp(-(loc * loc) / (2.0 * s * s + np.float32(EPS))
               - np.log(np.sqrt(np.float32(2.0 * np.pi)) * s + np.float32(EPS)))
    g = g.astype(np.float32)
    return g / g.sum(dtype=np.float32)


def _band(g):
    # T[r, c] = g[r - c + HALF] on the band, zero elsewhere ('SAME' zero pad)
    t = np.zeros((N, N), np.float32)
    for k in range(NB):
        off = k - HALF  # r = c + off
        c0 = max(0, -off)
        c1 = min(N, N - off)
        idx = np.arange(c0, c1)
        t[idx + off, idx] = g[k]
    return t


def _gpack(sigma_row):
    """[128, G_COLS] bf16: cols [p*134,(p+1)*134) = T_p[0:128, 0:134];
    cols [402+38p : 402+38(p+1)) = T_p[128:160, 122:160] replicated on
    all four 32-partition quadrants."""
    out = np.zeros((128, G_COLS), np.float32)
    for p in range(3):
        t = _band(_gaussian_1d(sigma_row[p]))
        out[:, p * A_N:(p + 1) * A_N] = t[0:128, 0:A_N]
        gb = t[128:N, B_C0:N]
        for m in range(4):
            out[32 * m:32 * (m + 1), GB_C0 + p * B_N:GB_C0 + (p + 1) * B_N] = gb
    return out.astype(BF16)


class _Emitter:
    """Round-robin psum->SBUF copies between DVE and ACT."""

    def __init__(self, nc):
        self.nc = nc
        self.flip = 0

    def copy(self, dst, src):
        if self.flip % 2 == 0:
            self.nc.vector.tensor_copy(dst, src)
        else:
            self.nc.scalar.copy(dst, src)
        self.flip += 1


def _emit_bank(nc, mms):
    """mms: ordered list of (out_ap, lhsT, rhs, tile_position) written to
    ONE psum bank. start=True zeroes the 2KB bank region ONLY on the
    partitions the matmul writes, so the first MM touching each partition
    range carries start; emission order must make that first MM cover the
    range fully (bigs before tails). Last MM gets stop=True."""
    n = len(mms)
    started = [False] * 128
    for i, (out, lhsT, rhs, tp) in enumerate(mms):
        b = tp[1]  # out base partition == tile col position (can't call
        m = out.partition_size()  # base_partition(): it asserts on 96)
        start = not any(started[b:b + m])
        if start:
            for p in range(b, b + m):
                started[p] = True
        nc.tensor.matmul(out, lhsT, rhs, start=start, stop=(i == n - 1),
                         skip_group_check=True, tile_position=tp)


def _build_kernel(ctx, tc, xm_in, xt_in, g_in, y_out):
    nc = tc.nc
    em = _Emitter(nc)

    gpool = ctx.enter_context(tc.tile_pool(name="g", bufs=1))
    big = ctx.enter_context(tc.tile_pool(name="big", bufs=1))
    stage = ctx.enter_context(tc.tile_pool(name="st", bufs=3))
    ps1 = ctx.enter_context(tc.tile_pool(name="ps1", bufs=5, space="PSUM"))
    ps2 = ctx.enter_context(tc.tile_pool(name="ps2", bufs=2, space="PSUM"))

    gtile = gpool.tile([128, G_COLS], BF)
    nc.sync.dma_start(gtile[:], g_in)

    def ga(p):  # [128, 134] rhs, base partition 0
        return gtile[:, (p - 1) * A_N:p * A_N]

    def gb(p, m):  # [32, 38] rhs at partition quadrant m
        return gtile[32 * m:32 * (m + 1),
                     GB_C0 + (p - 1) * B_N:GB_C0 + p * B_N]

    # persistent volume tiles
    x0m = big.tile([128, S], BF, tag="sA")    # x0 main rows d' 0:128, (w,h)
    x1m = big.tile([128, S], BF, tag="sB")    # x1 main rows h' 0:128, (w,d)
    # tailsA: parts[0:32) = x0 tail rows d'' (w,h) all cols;
    #         parts[32:64) = x1 corner (h'' x w'') cols [0:5120) = (w'', d)
    tailsA = big.tile([128, S], BF, tag="sT")
    # x1 tail rows h'', striped: part 32j+h'' , col (w%32)*160+d, j = w//32
    tailsB = big.tile([128, 32 * N], BF, tag="sU")
    # x2 tail rows w'', striped: part 32(d%4)+w'', col (d//4)*160+h
    x2t = big.tile([128, 40 * N], BF, tag="sX")

    # ---- stream input (bf16, no cast needed)
    for g0 in range(0, S, SLAB):
        sl = min(SLAB, S - g0)
        nc.sync.dma_start(x0m[:, g0:g0 + sl], xm_in[:, g0:g0 + sl])
        nc.sync.dma_start(tailsA[0:32, g0:g0 + sl], xt_in[:, g0:g0 + sl])

    # ================= pass 1: conv along d, chunks = fixed w =============
    def p1_pa_group(w0, glen):
        # one psum BANK per <=3 chunks; copy straight out (deep psum pipeline)
        pa = ps1.tile([128, BANK], F32, tag="pa", name=f"p1a_{w0}",
                      uniquify=True)
        mms = []
        for j in range(glen):
            o = j * N
            w = w0 + j
            mms.append((pa[0:128, o:o + A_N],
                        x0m[:, w * N:w * N + 128], ga(1), (0, 0)))
        for j in range(glen):
            o = j * N
            w = w0 + j
            mms.append((pa[0:128, o + B_C0:o + N],
                        tailsA[0:32, w * N:w * N + 128], gb(1, 0), (0, 0)))
        _emit_bank(nc, mms)
        em.copy(x1m[:, w0 * N:(w0 + glen) * N], pa[0:128, 0:glen * N])

    def p1_pb_bank(r0, rl):
        # rounds r0..r0+rl-1; round r packs w = 32j+r (j=0..3) into col grps
        pb = ps2.tile([128, BANK], F32, tag="pb", name=f"p1b_{r0}",
                      uniquify=True)
        mms = []
        for r in range(r0, r0 + rl):
            o = (r - r0) * N
            for j in range(4):
                w = 32 * j + r
                mms.append((pb[32 * j:32 * j + 32, o:o + A_N],
                            x0m[:, w * N + 128:w * N + N], ga(1), (0, 32 * j)))
            for j in range(4):
                w = 32 * j + r
                mms.append((pb[32 * j:32 * j + 32, o + B_C0:o + N],
                            tailsA[0:32, w * N + 128:w * N + N], gb(1, 0),
                            (0, 32 * j)))
        _emit_bank(nc, mms)
        em.copy(tailsB[:, r0 * N:(r0 + rl) * N], pb[0:128, 0:rl * N])

    def p1_corner_bank(c0, cl):
        # chunks w = 128+c (c0..c0+cl-1): out h'' tail of w-tail columns
        pc = ps2.tile([128, BANK], F32, tag="pb", name=f"p1c_{c0}",
                      uniquify=True)
        mms = []
        for c in range(c0, c0 + cl):
            o = (c - c0) * N
            w = 128 + c
            mms.append((pc[32:64, o:o + A_N],
                        x0m[:, w * N + 128:w * N + N], ga(1), (0, 32)))
            mms.append((pc[32:64, o + B_C0:o + N],
                        tailsA[0:32, w * N + 128:w * N + N], gb(1, 0),
                        (0, 32)))
        _emit_bank(nc, mms)
        em.copy(tailsA[32:64, c0 * N:(c0 + cl) * N], pc[32:64, 0:cl * N])

    pb_next = 0    # 32 rounds total, banks of 3
    cor_next = 0   # 32 corner chunks, banks of 3
    for w0 in range(0, N, 3):
        glen = min(3, N - w0)
        p1_pa_group(w0, glen)
        w_end = w0 + glen - 1
        # pb bank (r0..r0+2) feasible once w = 96+r0+2 is loaded/used
        while pb_next < 32 and 96 + min(pb_next + 2, 31) <= w_end:
            rl = min(3, 32 - pb_next)
            p1_pb_bank(pb_next, rl)
            pb_next += rl
        while cor_next < 32 and 128 + min(cor_next + 2, 31) <= w_end:
            cl = min(3, 32 - cor_next)
            p1_corner_bank(cor_next, cl)
            cor_next += cl
    while pb_next < 32:
        rl = min(3, 32 - pb_next)
        p1_pb_bank(pb_next, rl)
        pb_next += rl
    while cor_next < 32:
        cl = min(3, 32 - cor_next)
        p1_corner_bank(cor_next, cl)
        cor_next += cl

    if _VARIANT == "p1":
        nc.sync.dma_start(y_out[0:128, :], x1m[:, 0:N])
        return

    # x2 main reuses x0's slot (x0 fully consumed by pass 1)
    x2m = big.tile([128, S], BF, tag="sA")

    # ================= pass 2: conv along h, chunks = fixed d =============
    x1v = x1m[:].rearrange("p (w d) -> p w d", d=N)
    tBv = tailsB[:].rearrange("p (s d) -> p s d", d=N)
    cnv = tailsA[:].rearrange("p (s d) -> p s d", d=N)  # corner view

    def p2_pa_group(d0, glen):
        pa = ps1.tile([128, BANK], F32, tag="pa", name=f"p2a_{d0}",
                      uniquify=True)
        mms = []
        for j in range(glen):
            o = j * N
            d = d0 + j
            mms.append((pa[0:128, o:o + A_N],
                        x1v[:, 0:128, d], ga(2), (0, 0)))
        for j in range(glen):
            o = j * N
            d = d0 + j
            for m in range(4):
                mms.append((pa[32 * m:32 * m + 32, o + B_C0:o + N],
                            tBv[32 * m:32 * m + 32, 0:32, d], gb(2, m),
                            (32 * m, 32 * m)))
        _emit_bank(nc, mms)
        em.copy(x2m[:, d0 * N:(d0 + glen) * N], pa[0:128, 0:glen * N])

    def p2_pb_bank(q0, ql):
        # quads q0..q0+ql-1; quad q packs d = 4q+j (j=0..3) into col grps
        pb = ps2.tile([128, BANK], F32, tag="pb", name=f"p2b_{q0}",
                      uniquify=True)
        mms = []
        for q in range(q0, q0 + ql):
            o = (q - q0) * N
            for j in range(4):
                d = 4 * q + j
                mms.append((pb[32 * j:32 * j + 32, o:o + A_N],
                            x1v[:, 128:160, d], ga(2), (0, 32 * j)))
            for j in range(4):
                d = 4 * q + j
                mms.append((pb[32 * j:32 * j + 32, o + B_C0:o + N],
                            cnv[32:64, 0:32, d], gb(2, 1), (32, 32 * j)))
        _emit_bank(nc, mms)
        em.copy(x2t[:, q0 * N:(q0 + ql) * N], pb[0:128, 0:ql * N])

    # ================= pass 3: conv along w, chunks = 128-col (d,h) blocks
    x2tv = x2t[:].rearrange("p (q h) -> p q h", h=N)
    yv = y_out.rearrange("(k p) w -> p k w", p=128)
    NK = S // 128  # 200

    def p3_group(k0, glen):
        # NOTE: all tail pieces are M=32: two adjacent K=32 matmuls at
        # different row groups with M>=64 hard-crash the PE (LDWEIGHTS
        # pull-ahead across row groups); M=32 cross-row adjacency is the
        # pattern pass-2's diagonal tails use in production and is safe.
        ps = ps1.tile([128, BANK], F32, tag="pa", name=f"p3_{k0}",
                      uniquify=True)
        mms = []
        for j in range(glen):
            o = j * N
            k = k0 + j
            mms.append((ps[0:128, o:o + A_N],
                        x2m[:, k * 128:k * 128 + 128], ga(3), (0, 0)))
            c = k * 128
            for pa_ in range(0, 128, 32):
                d = (c + pa_) // N
                m = d % 4
                q = d // 4
                hh = c + pa_ - d * N
                mms.append((ps[pa_:pa_ + 32, o + B_C0:o + N],
                            x2tv[32 * m:32 * m + 32, q, hh:hh + 32],
                            gb(3, m), (32 * m, pa_)))
        _emit_bank(nc, mms)
        st = stage.tile([128, 3 * N], BF, tag="st", name=f"st_{k0}",
                        uniquify=True)
        em.copy(st[:, 0:glen * N], ps[0:128, 0:glen * N])
        stv = st[:].rearrange("p (k w) -> p k w", w=N)
        nc.sync.dma_start(yv[0:128, k0:k0 + glen, :], stv[:, 0:glen, :])

    # pass 2 with pass-3 groups interleaved as soon as their x2 cols exist
    p3_enabled = _VARIANT not in ("p2",)
    q_next = 0   # pb2 quads (40), banks of 3
    k_next = 0   # pass-3 blocks (200), banks of 3
    for d0 in range(0, N, 3):
        glen = min(3, N - d0)
        p2_pa_group(d0, glen)
        d_end = d0 + glen - 1
        while q_next < 40 and 4 * min(q_next + 2, 39) + 3 <= d_end:
            ql = min(3, 40 - q_next)
            p2_pb_bank(q_next, ql)
            q_next += ql
        while p3_enabled and k_next < NK:
            gl = min(3, NK - k_next)
            dmax = (128 * (k_next + gl) - 1) // N
            if dmax <= d_end and dmax <= 4 * q_next - 1:
                p3_group(k_next, gl)
                k_next += gl
            else:
                break
    while q_next < 40:
        ql = min(3, 40 - q_next)
        p2_pb_bank(q_next, ql)
        q_next += ql
    if not p3_enabled:
        nc.sync.dma_start(y_out[0:128, :], x2m[:, 0:N])
        return
    while k_next < NK:
        gl = min(3, NK - k_next)
        p3_group(k_next, gl)
        k_next += gl


def _build_program():
    global _PROGRAM
    if _PROGRAM is not None:
        return _PROGRAM
    nc = bacc.Bacc("TRN2", target_bir_lowering=False, debug=False,
                   num_devices=8)
    xm_in = nc.dram_tensor("xm_in", [128, S], BF, kind="ExternalInput").ap()
    xt_in = nc.dram_tensor("xt_in", [32, S], BF, kind="ExternalInput").ap()
    g_in = nc.dram_tensor("g_in", [128, G_COLS], BF, kind="ExternalInput").ap()
    y_out = nc.dram_tensor("y_out", [S, N], BF, kind="ExternalOutput").ap()
    with tile.TileContext(nc) as tc, ExitStack() as ctx:
        _build_kernel(ctx, tc, xm_in, xt_in, g_in, y_out)
    nc.compile()
    _PROGRAM = nc
    return nc


def _run(image, sigma, **spmd_kwargs):
    nc = _build_program()
    B, _, _, _, C = image.shape
    in_maps = []
    for core in range(8):
        b, c = divmod(core, C)
        # (d, h, w) -> (d, w, h), bf16
        vol = np.ascontiguousarray(
            image[b, :, :, :, c].transpose(0, 2, 1)).astype(BF16)
        in_maps.append({
            "xm_in": np.ascontiguousarray(vol[0:128].reshape(128, S)),
            "xt_in": np.ascontiguousarray(vol[128:N].reshape(32, S)),
            "g_in": _gpack(sigma[b]),
        })
    res = run_bass_kernel_spmd(nc, in_maps, list(range(8)), **spmd_kwargs)
    out = np.empty((B, N, N, N, C), np.float32)
    for core in range(8):
        b, c = divmod(core, C)
        y = res.results[core]["y_out"].astype(np.float32)
        out[b, :, :, :, c] = y.reshape(N, N, N)
    return out, res


def kernel(image, sigma):
    image = np.asarray(image, dtype=np.float32)
    sigma = np.asarray(sigma, dtype=np.float32)
    out, _ = _run(image, sigma)
    return out
